# revision 6
# baseline (speedup 1.0000x reference)
"""Chamfer-distance (CDLoss) kernel for 8x TRN2 NeuronCores.

Strategy
--------
Data-parallel over batch: core b handles batch b (B=8).

Both clouds are z-sorted on host (the chamfer mean is permutation
invariant).  Phase 1 computes, for each 128-query block, the min squared
distance to a per-block candidate window (quad-level width schedule WQ,
per-block placement CQ, derived offline for the fixed problem
distribution).  A certificate (window min <= margin^2, margin = z-gap to
the nearest live window edge) proves per-query exactness; failures are
repaired exactly in phase 2 by scanning all M candidates.

Engine split (derived from measured op rates):
  - TensorE: distances via K=7 fp16 Gram matmul into PSUM quads
    [128, 4, 512] (one bank per block).
  - DVE: tensor_reduce (min) over the first Wd = W - WA columns of each
    quad directly from PSUM (1 elem/cycle).
  - ACT: copies the last WA columns of each block PSUM->SBUF fp16 into a
    packed [128, 64, WA] tile (this offloads the DVE).
  - DVE: folds the packed tile with a log-depth tensor_tensor min tree
    (fp16 SBUF runs at 2 elem/cycle) -- a handful of ops per direction
    covers all 64 blocks, then one small reduce + a combine with the
    direct minima.

Phase 2 uses the same split on the full-scan repair (CAP=128 queries per
direction vs all M candidates).

The squared distances come from the Gram expansion packed into a K=7
fp16 matmul:  d[n,m] = |x_n|^2 + |y_m|^2 - 2 x_n.y_m
    lhsT rows: [nhi_x, nlo_x, 1, 1, -2x0, -2x1, -2x2]
    rhs  rows: [1, 1, nhi_y, nlo_y, y0, y1, y2]
Squared norms are hi/lo split across two fp16 rows; PSUM accumulates in
fp32.  fp16 quantisation perturbs d by ~1e-4 absolute, absorbed by
CERT_SLACK in the certificate test.
"""

import numpy as np

try:
    import concourse.bass as bass  # noqa: F401
except ImportError:  # harness environments without concourse on sys.path
    import sys

    sys.path.insert(0, "/opt/trn_rl_repo")

import concourse.bass as bass
import concourse.tile as tile
from concourse import mybir
from concourse.bass_utils import run_bass_kernel_spmd

B, N, M = 8, 8192, 8192
K = 7  # Gram-expansion contraction dim
NB = N // 128  # query blocks per batch
NQ = NB // 4  # quads per direction
CAP = 128  # phase-2 repair queries per direction
MT = M // 512  # phase-2 candidate tiles per direction
CERT_SLACK = 3e-4  # fp16 distance noise absorbed into the certificate test
N_CORES = 8
WA = 224  # per-block columns routed via ACT->fp16 tree
WA2 = 256  # phase-2 per-tile columns routed via ACT->fp16 tree

# Quad-level window widths + per-block placements (offline, fixed input
# distribution; see module docstring).  WQ[d][q] is the width shared by
# blocks 4q..4q+3 of direction d; CQ[d][q][j] is block 4q+j's window start.
WQ = {0: [288, 288, 320, 352, 448, 416, 480, 512, 480, 448, 480, 480, 416, 352, 320, 256], 1: [256, 288, 352, 416, 480, 416, 480, 512, 480, 448, 480, 512, 384, 352, 320, 256]}
CQ = {0: [[0, 80, 204, 326], [466, 572, 705, 826], [933, 1080, 1193, 1327], [1442, 1568, 1684, 1819], [1909, 2019, 2163, 2285], [2409, 2544, 2670, 2802], [2918, 2992, 3151, 3280], [3381, 3492, 3613, 3764], [3888, 4035, 4165, 4297], [4435, 4558, 4701, 4840], [4958, 5072, 5212, 5329], [5438, 5576, 5702, 5841], [5978, 6102, 6244, 6383], [6519, 6646, 6787, 6929], [7064, 7194, 7329, 7464], [7600, 7743, 7878, 7936]], 1: [[0, 92, 207, 318], [437, 553, 676, 797], [913, 1032, 1162, 1285], [1398, 1522, 1649, 1773], [1883, 2005, 2140, 2262], [2397, 2529, 2651, 2777], [2895, 2984, 3143, 3274], [3371, 3479, 3604, 3754], [3877, 4024, 4152, 4284], [4424, 4548, 4690, 4829], [4947, 5061, 5203, 5318], [5424, 5561, 5688, 5826], [5975, 6100, 6246, 6388], [6524, 6652, 6793, 6935], [7069, 7198, 7333, 7468], [7603, 7745, 7879, 7936]]}

_SCHED_OK = all(
    WA + 32 <= WQ[d][q] <= 512 and 0 <= CQ[d][q][j] <= M - WQ[d][q]
    for d in range(2) for q in range(NQ) for j in range(4)
)
assert _SCHED_OK


def _forms(p):
    """fp16 lhsT/rhs Gram forms for one sorted cloud p [n, 3] fp32."""
    q = p.astype(np.float16)
    qf = q.astype(np.float32)
    nrm = (qf * qf).sum(-1)
    nh = nrm.astype(np.float16)
    nl = (nrm - nh.astype(np.float32)).astype(np.float16)
    one = np.ones_like(nh)
    lhsT = np.stack([nh, nl, one, one, -2 * q[:, 0], -2 * q[:, 1], -2 * q[:, 2]])
    rhs = np.stack([one, one, nh, nl, q[:, 0], q[:, 1], q[:, 2]])
    return lhsT, rhs


def _elide_redundant_waits(nc):
    """Drop transitively-redundant sem waits so every instruction has <=1.

    The walrus build in this image rejects instructions carrying more than
    one sync wait ("Too many sync wait commands").  Tile emits per-proc
    minimal waits but not transitively-minimal ones: e.g. a matmul that
    waits on both "my own earlier matmuls completed" (PE sem) and "the DVE
    reduce of those matmuls completed" (DVE sem) — the DVE wait implies
    the PE wait, because the reduce itself waited on those matmuls.

    We compute, per instruction in committed (scheduled) order, the
    vector-clock of sem values each engine has provably observed —
    inheriting the updater's clock when waiting on a semaphore — and drop
    any wait implied by another wait on the same instruction or already
    observed by the engine.  Leftover multi-waits are hoisted onto NoOps.
    """
    import copy as _copy

    blocks = nc.m.functions[0].blocks
    insts = [i for blk in blocks for i in blk.instructions]
    loc = {}
    for blk in blocks:
        for i in blk.instructions:
            loc[i.name] = blk
    obs = {}  # engine -> {sem: value observed}
    cum = {}  # sem -> cumulative update value
    snaps = {}  # sem -> list of (cum_value, snapshot dict) at each update

    def snap_at(sem, val):
        for cv, snap in snaps.get(sem, ()):
            if cv >= val:
                return snap
        return None

    for inst in insts:
        si = inst.sync_info
        eng = inst.engine
        o = obs.setdefault(eng, {})
        if si and si.on_wait:
            waits = list(si.on_wait)
            kept = list(waits)
            changed = True
            while changed and len(kept) > 1:
                changed = False
                for k, w in enumerate(kept):
                    others = kept[:k] + kept[k + 1 :]
                    imp = o.get(w.ant_name, 0) >= w.wait_value
                    for w2 in others:
                        if imp:
                            break
                        if w2.ant_name == w.ant_name and w2.wait_value >= w.wait_value:
                            imp = True
                            break
                        snap = snap_at(w2.ant_name, w2.wait_value)
                        if snap is not None and snap.get(w.ant_name, 0) >= w.wait_value:
                            imp = True
                    if imp:
                        kept.pop(k)
                        changed = True
                        break
            if len(kept) > 1:
                blk = loc[inst.name]
                pos = next(
                    k for k, i2 in enumerate(blk.instructions) if i2.name == inst.name
                )
                for j, w in enumerate(kept[:-1]):
                    nop = mybir.InstNoOp(name=f"{inst.name}-hw{j}", ins=[], outs=[])
                    nop.engine = eng
                    nsi = _copy.deepcopy(si)
                    nsi.on_wait[:] = [w]
                    if nsi.on_update:
                        nsi.on_update[:] = []
                    nop.sync_info = nsi
                    blk.instructions.insert(pos + j, nop)
                kept = kept[-1:]
            si.on_wait[:] = kept
            for w in waits:
                if o.get(w.ant_name, 0) < w.wait_value:
                    o[w.ant_name] = w.wait_value
                snap = snap_at(w.ant_name, w.wait_value)
                if snap is not None:
                    for s, v in snap.items():
                        if o.get(s, 0) < v:
                            o[s] = v
        if si and si.on_update:
            for u in si.on_update:
                name = u.ant_name
                inc = getattr(u, "value", None) or getattr(u, "update_value", None)
                if inc is None:
                    inc = 16 if name.startswith("DMA") else 1
                cum[name] = cum.get(name, 0) + inc
                snaps.setdefault(name, []).append((cum[name], dict(o)))


def _fold_tree(nc, pck, tmp, mins_out, nblk, wa):
    """DVE fp16 min-fold of pck [128, nblk, wa] -> mins_out [128, nblk]."""
    f16 = mybir.dt.float16
    MIN = mybir.AluOpType.min
    X = mybir.AxisListType.X
    src = pck
    w = wa
    off = 0
    while w > 8:
        h = w // 2
        dst = tmp  # tmp is [128, nblk, >= wa/2]; reuse disjoint column bands
        nc.vector.tensor_tensor(
            dst[:, :, off : off + h], src[:, :, :h], src[:, :, h : h + h], MIN
        )
        if w % 2:  # fold the odd straggler column into the first output col
            nc.vector.tensor_tensor(
                dst[:, :, off : off + 1], dst[:, :, off : off + 1],
                src[:, :, w - 1 : w], MIN,
            )
        src = dst[:, :, off : off + h]
        off += h
        w = h
    nc.vector.tensor_reduce(mins_out, src, axis=X, op=MIN)


def _build_phase1():
    f16, f32 = mybir.dt.float16, mybir.dt.float32
    X, MIN = mybir.AxisListType.X, mybir.AluOpType.min
    COPY = mybir.ActivationFunctionType.Copy

    nc = bass.Bass()
    # pts[:, 0]=lhsT(x), 1=rhs(y), 2=lhsT(y), 3=rhs(x); all z-sorted
    pts = nc.declare_dram_parameter("pts", [K, 4, N], f16, isOutput=False)
    mins = nc.declare_dram_parameter("mins", [128, 2, NB], f32, isOutput=True)

    with tile.TileContext(nc) as tc:
        with (
            tc.tile_pool(name="singles", bufs=1) as singles,
            tc.tile_pool(name="psum", bufs=2, space="PSUM") as psum,
        ):
            P = singles.tile([K, 4, N], f16)
            Q4 = N // 4
            for cp in (0, 2):
                for q in range(4):
                    nc.sync.dma_start(
                        out=P[:, cp : cp + 2, q * Q4 : (q + 1) * Q4],
                        in_=pts[:, cp : cp + 2, q * Q4 : (q + 1) * Q4],
                    )
            mtd = singles.tile([128, 2, NB], f32)  # direct-part minima
            mt = singles.tile([128, 2, NB], f32)  # final minima
            pck = singles.tile([128, 2, NB, WA], f16)  # ACT-staged columns
            tmp = singles.tile([128, NB, WA], f16)  # fold-tree scratch

            for d in range(2):
                for g in range(NQ):
                    W = WQ[d][g]
                    Wd = W - WA
                    pt = psum.tile([128, 4, 512], f32, tag="pt")
                    for j in range(4):
                        blk = 4 * g + j
                        c = CQ[d][g][j]
                        nc.tensor.matmul(
                            pt[:, j, :W],
                            P[:, 2 * d, 128 * blk : 128 * blk + 128],
                            P[:, 2 * d + 1, c : c + W],
                            start=True,
                            stop=True,
                        )
                        nc.scalar.activation(
                            pck[:, d, blk, :], pt[:, j, Wd:W], COPY
                        )
                    nc.vector.tensor_reduce(
                        mtd[:, d, 4 * g : 4 * g + 4],
                        pt[:, :, :Wd],
                        axis=X,
                        op=MIN,
                    )
                # fold the ACT-staged columns of this direction
                _fold_tree(nc, pck[:, d], tmp, mt[:, d, :], NB, WA)
                # combine with the direct minima
                nc.vector.tensor_tensor(mt[:, d, :], mt[:, d, :], mtd[:, d, :], MIN)
                nc.sync.dma_start(out=mins[:, d, :], in_=mt[:, d, :])

    _elide_redundant_waits(nc)
    return nc


def _build_phase2():
    f16, f32 = mybir.dt.float16, mybir.dt.float32
    X, MIN = mybir.AxisListType.X, mybir.AluOpType.min
    COPY = mybir.ActivationFunctionType.Copy

    nc = bass.Bass()
    q2 = nc.declare_dram_parameter("q2", [K, 2, CAP], f16, isOutput=False)
    cand = nc.declare_dram_parameter("cand", [K, 2, M], f16, isOutput=False)
    mins2 = nc.declare_dram_parameter("mins2", [CAP, 2, 5], f32, isOutput=True)

    with tile.TileContext(nc) as tc:
        with (
            tc.tile_pool(name="singles", bufs=1) as singles,
            tc.tile_pool(name="psum", bufs=2, space="PSUM") as psum,
        ):
            Q = singles.tile([K, 2, CAP], f16)
            nc.sync.dma_start(out=Q[:, :, :], in_=q2[:, :, :])
            C = singles.tile([K, 2, M], f16)
            Q4 = M // 4
            for dd in (0, 1):
                for q in range(4):
                    nc.sync.dma_start(
                        out=C[:, dd, q * Q4 : (q + 1) * Q4],
                        in_=cand[:, dd, q * Q4 : (q + 1) * Q4],
                    )
            mt = singles.tile([CAP, 2, 5], f32)  # 4 direct quads + 1 tree min
            pck = singles.tile([CAP, 2, MT, WA2], f16)
            tmp = singles.tile([CAP, MT, WA2], f16)
            W2d = 512 - WA2

            for d in range(2):
                for g in range(MT // 4):
                    pt = psum.tile([CAP, 4, 512], f32, tag="pt")
                    for j in range(4):
                        jj = 4 * g + j
                        nc.tensor.matmul(
                            pt[:, j, :],
                            Q[:, d, :],
                            C[:, d, 512 * jj : 512 * jj + 512],
                            start=True,
                            stop=True,
                        )
                        nc.scalar.activation(
                            pck[:, d, jj, :], pt[:, j, W2d:], COPY
                        )
                    nc.vector.tensor_reduce(
                        mt[:, d, g : g + 1], pt[:, :, :W2d], axis=mybir.AxisListType.XY,
                        op=MIN,
                    )
                ftmp = singles.tile([CAP, MT], f16, tag=f"ftmp{d}")
                _fold_tree(nc, pck[:, d], tmp, ftmp, MT, WA2)
                nc.vector.tensor_reduce(
                    mt[:, d, 4:5], ftmp, axis=X, op=MIN
                )
                nc.sync.dma_start(out=mins2[:, d, :], in_=mt[:, d, :])

    _elide_redundant_waits(nc)
    return nc


def _install_ntff_hook():
    """Provide antenv.axon_hooks (absent in this image) so trace=True works."""
    import contextlib
    import ctypes
    import sys
    import types

    if "antenv.axon_hooks" in sys.modules:
        return
    hook = None
    try:
        lib = ctypes.CDLL("/opt/axon/libaxon_pjrt.so")
        if hasattr(lib, "axon_start_nrt_profile"):
            lib.axon_start_nrt_profile.argtypes = [
                ctypes.POINTER(ctypes.c_int64),
                ctypes.c_size_t,
            ]
            lib.axon_start_nrt_profile.restype = ctypes.c_int64
            lib.axon_stop_nrt_profile.argtypes = [ctypes.c_char_p]
            lib.axon_stop_nrt_profile.restype = ctypes.c_int64

            @contextlib.contextmanager
            def _hook(output_dir, device_ids):
                import jax

                jax.devices()
                if device_ids:
                    ids = (ctypes.c_int64 * len(device_ids))(*device_ids)
                    rc = lib.axon_start_nrt_profile(ids, len(device_ids))
                else:
                    rc = lib.axon_start_nrt_profile(None, 0)
                if rc != 0:
                    raise RuntimeError(f"axon_start_nrt_profile rc={rc}")
                try:
                    yield
                finally:
                    n = lib.axon_stop_nrt_profile(str(output_dir).encode())
                    print(f"profile: {n} file(s) written to {output_dir}")

            hook = _hook
    except OSError:
        pass

    mod = types.ModuleType("antenv.axon_hooks")
    mod.get_axon_ntff_profile_hook = lambda: hook
    mod.set_axon_ntff_profile_hook = lambda h: None
    sys.modules["antenv.axon_hooks"] = mod

    from concourse import bass_utils

    bass_utils.upload_artifacts = lambda tmpdir: f"local://{tmpdir}"


def _dump_insts(res, path):
    import pickle

    insts, _ = res.instructions_and_trace
    rows = []
    for inst in insts:
        row = {}
        for a in ("name", "engine", "duration", "timestamp", "end_timestamp",
                  "pc", "operands", "bir_instruction_name"):
            try:
                row[a] = getattr(inst, a)
            except Exception:
                pass
        rows.append(row)
    with open(path, "wb") as f:
        pickle.dump(rows, f)


def _cert(zq, zc, d):
    """Certificate bound per query rank for direction d's window schedule."""
    cert = np.empty(len(zq), np.float64)
    for blk in range(len(zq) // 128):
        W = WQ[d][blk // 4]
        c = CQ[d][blk // 4][blk % 4]
        xs = slice(128 * blk, 128 * blk + 128)
        lo = zq[xs] - zc[c] if c > 0 else np.full(128, np.inf)
        hi = zc[c + W - 1] - zq[xs] if c + W < len(zc) else np.full(128, np.inf)
        m = np.minimum(lo, hi)
        cert[xs] = np.where(m > 0, m * m, 0.0)
    return cert


def kernel(pcs1, pcs2, _trace=False):
    pcs1 = np.asarray(pcs1, dtype=np.float32)
    pcs2 = np.asarray(pcs2, dtype=np.float32)
    if _trace:
        _install_ntff_hook()

    batches = []  # (z1, z2, l1, r1, l2, r2) per batch, z-sorted
    in_maps1 = []
    for b in range(B):
        i1 = np.argsort(pcs1[b, :, 2], kind="stable")
        i2 = np.argsort(pcs2[b, :, 2], kind="stable")
        x = pcs1[b][i1]
        y = pcs2[b][i2]
        l1, r1 = _forms(x)
        l2, r2 = _forms(y)
        pts = np.stack([l1, r2, l2, r1], axis=1)
        in_maps1.append({"pts": np.ascontiguousarray(pts, dtype=np.float16)})
        batches.append(
            (x[:, 2].astype(np.float64), y[:, 2].astype(np.float64), l1, r1, l2, r2)
        )

    cores = list(range(N_CORES))
    res1 = run_bass_kernel_spmd(_build_phase1(), in_maps1, cores, trace=_trace)
    t1 = res1.exec_time_ns
    if _trace and res1.instructions_and_trace:
        _dump_insts(res1, "/root/problem/p1_insts.pkl")
    if _trace:
        np.save("/root/problem/p1_mins.npy",
                np.stack([np.asarray(res1.results[b]["mins"]) for b in range(B)]))

    # certificate check -> phase-2 query selection
    fails_all = []  # [b][d] -> rank indices needing exact repair
    vals_all = []  # [b][d] -> rank-ordered window minima
    nrounds = 1
    for b in range(B):
        z1, z2, l1, r1, l2, r2 = batches[b]
        mtv = np.asarray(res1.results[b]["mins"], dtype=np.float64)  # [128, 2, 64]
        dir_fails = []
        dir_vals = []
        for d, (zq, zc) in enumerate(((z1, z2), (z2, z1))):
            wmins = mtv[:, d, :].T.reshape(-1)  # rank-ordered window minima
            fails = np.where(wmins > _cert(zq, zc, d) - CERT_SLACK)[0]
            nrounds = max(nrounds, -(-len(fails) // CAP))
            dir_fails.append(fails)
            dir_vals.append(wmins.copy())
        fails_all.append(dir_fails)
        vals_all.append(dir_vals)

    # phase-2 exact repair; multiple rounds if >CAP queries fail anywhere
    nc2 = _build_phase2()
    t2 = 0
    for rnd in range(nrounds):
        in_maps2 = []
        for b in range(B):
            _, _, l1, r1, l2, r2 = batches[b]
            qsel = np.zeros((K, 2, CAP), np.float16)
            qsel[2:4, :, :] = 1.0  # harmless queries (|q|=0 rows stay 0)
            for d, lq in enumerate((l1, l2)):
                fl = fails_all[b][d][rnd * CAP : (rnd + 1) * CAP]
                if len(fl):
                    qsel[:, d, : len(fl)] = lq[:, fl]
            in_maps2.append(
                {
                    "q2": qsel,
                    "cand": np.ascontiguousarray(np.stack([r2, r1], axis=1), np.float16),
                }
            )
        res2 = run_bass_kernel_spmd(nc2, in_maps2, cores, trace=_trace)
        if _trace and res2.exec_time_ns is not None:
            t2 += res2.exec_time_ns
        if _trace and res2.instructions_and_trace:
            _dump_insts(res2, "/root/problem/p2_insts.pkl")
        for b in range(B):
            m2 = np.asarray(res2.results[b]["mins2"], dtype=np.float64).min(-1)
            for d in range(2):
                fl = fails_all[b][d][rnd * CAP : (rnd + 1) * CAP]
                vals_all[b][d][fl] = m2[: len(fl), d]

    if _trace and t1 is not None:
        print(f"HW exec time: {t1 + t2} ns (phase1 {t1} + phase2 {t2} x{nrounds})")

    total = np.float64(0.0)
    for b in range(B):
        for d in range(2):
            total += np.maximum(vals_all[b][d], 0.0).sum()
    return np.float32(total / (B * N))
